# revision 73
# baseline (speedup 1.0000x reference)
"""Distributed Bass kernel: fused multi-head attention block on 8 TRN2 NeuronCores.

Problem: x[2,2048,1024] -> QKV proj -> RoPE(q,k) -> softmax(q k^T/8) v -> out proj.

Sharding: tensor-parallel over heads. 16 heads / 8 cores = 2 heads per core.
Each core computes QKV for its 2 heads (full sequence), RoPE, attention, then
chunked AllToAlls (4 pieces of 1024 tokens, cc_dim=Free, pipelined under the
attention compute) convert head-sharding to token-sharding so the output
projection runs against the FULL Wout with no AllReduce. Token ownership is
interleaved per 128-token tile: within piece p core j owns tokens
[p*1024+j*128, p*1024+(j+1)*128). Host reassembles the 4x128-row tiles.

Schedule (v2): all bulk loads are single large DMA descriptors (host
pre-swizzles x/Wqkv/Wout so each is one contiguous block per chunk) issued
from the sync/scalar HWDGE rings instead of gpsimd SWDGE, which cuts the
descriptor-issue preamble from ~20us to ~6us. QKV chunks 4-7 are interleaved
per-PAIR into attention chunks 0-3 so the PE never stalls on the scalar
engine's exp stream. The final chunk's PV pairs interleave with its own exp
stream, with outproj(2) as PE filler, then normalize->stage->AllToAll->
outproj for the last piece runs immediately.

Compute dtype bf16, f32 PSUM accumulation. Softmax skips the max-subtraction
(scores ~N(0,2), exp safe in f32) and folds the denominator into the PV matmul
via a ones-column in the per-head v table ([key,130] slots: vA|1|vB|1).
"""

import sys

for _p in ("/opt/trn_rl_repo", "/root/.axon_site/_ro/trn_rl_repo"):
    if _p not in sys.path:
        sys.path.append(_p)

import numpy as np
import ml_dtypes

B, N, HID = 2, 2048, 1024
H, DH = 16, 64
NCORES = 8
HPC = H // NCORES          # heads per core = 2
T = B * N                  # 4096 flattened tokens
TS = T // NCORES           # 512 tokens per core after AllToAll
EPC = HPC * DH             # 128 features per core
CH = 512                   # token chunk for QKV phase
NCH = T // CH              # 8 chunks
KT = 128                   # key tile
QC = 512                   # query chunk in attention
NPIECE = 4                 # a2a pieces (1024 tokens each)
PT = T // NPIECE           # 1024 tokens per piece
VS = 2 * (DH + 1)          # 130-wide v-table slot: [vA(64) | 1 | vB(64) | 1]

_bf16 = ml_dtypes.bfloat16


def _build_graph():
    import concourse.bass as bass
    import concourse.mybir as mybir
    import concourse.tile as tile
    from concourse import bacc

    f32 = mybir.dt.float32
    bf16 = mybir.dt.bfloat16

    nc = bacc.Bacc("TRN2", target_bir_lowering=False, debug=False, num_devices=NCORES)

    # host pre-swizzled: xS[c*128+p, kt*512+t] = x^T[kt*128+p, c*512+t]
    xS_e = nc.declare_dram_parameter("xS", [HID, T], bf16, isOutput=False)
    # wqkvS[p, kt*384+j] = Wqkv'[kt*128+p, j]
    wqkvS_e = nc.declare_dram_parameter("wqkvS", [128, 8 * 3 * EPC], bf16, isOutput=False)
    # woutS[p, kt*1024+j] = Wout^T[kt*128+p, j]
    woutS_e = nc.declare_dram_parameter("woutS", [128, 8 * HID], bf16, isOutput=False)
    cos2_e = nc.declare_dram_parameter("cos2", [2 * DH, T], bf16, isOutput=False)
    sin2_e = nc.declare_dram_parameter("sin2", [2 * DH, T], bf16, isOutput=False)
    permident_e = nc.declare_dram_parameter("permident", [128, 256], bf16, isOutput=False)
    out_e = nc.declare_dram_parameter("out", [TS, HID], f32, isOutput=True)

    with tile.TileContext(nc) as tc:
        with (
            tc.tile_pool(name="const", bufs=1) as cpool,
            tc.tile_pool(name="work", bufs=1) as wpool,
            tc.tile_pool(name="stream", bufs=4) as spool,
            tc.tile_pool(name="psum", bufs=2, space="PSUM") as pspool,
            tc.tile_pool(name="dram", bufs=1, space="DRAM") as dpool,
        ):
            # ---- constants / weights (few-descriptor loads, HWDGE) ----
            # wqkv in halves so the first QKV matmuls start sooner
            wqkvT = cpool.tile([128, 8 * 3 * EPC], bf16)
            nc.sync.dma_start(wqkvT[:, 0:4 * 3 * EPC], wqkvS_e[:, 0:4 * 3 * EPC])
            nc.sync.dma_start(wqkvT[:, 4 * 3 * EPC:], wqkvS_e[:, 4 * 3 * EPC:])
            permident = cpool.tile([128, 256], bf16)
            nc.scalar.dma_start(permident[:, :], permident_e[:, :])
            perm = permident[:, 0:128]
            ident = permident[:, 128:256]
            woutT = cpool.tile([128, 8 * HID], bf16)
            cos2 = cpool.tile([128, T], bf16)
            sin2 = cpool.tile([128, T], bf16)

            # ---- persistent working tensors ----
            q_sb = wpool.tile([128, T], bf16)      # raw q (rope intermediate)
            k_sb = wpool.tile([128, T], bf16)      # becomes roped k
            qzA = wpool.tile([128, T], bf16)       # roped qA rows 0-63, 0 below
            qzB = wpool.tile([128, T], bf16)       # roped qB rows 64-127, 0 above
            vtab = wpool.tile([128, 32 * VS], bf16)  # [key, vA|1|vB|1] per slot
            ovT = wpool.tile([128, T], bf16)       # attention out ^T

            vt3 = vtab.rearrange("p (s c) -> p s c", c=VS)
            ones64 = cpool.tile([1, DH], bf16)
            nc.vector.memset(ones64[0:1, :], 1.0)
            nc.vector.memset(qzA[DH:128, :], 0.0)
            nc.vector.memset(qzB[0:DH, :], 0.0)
            nc.vector.memset(vt3[:, :, DH:DH + 1], 1.0)
            nc.vector.memset(vt3[:, :, 2 * DH + 1:2 * DH + 2], 1.0)

            # ================= Phase 1 chunk: QKV + RoPE + v-transpose ============
            # emitted as a generator of "pieces" so attention chunks can
            # interleave QKV work into their pair loops (fills exp-wait gaps)
            def emit_rope(sl, srd, dests):
                # RoPE: t = P@x * sin2 ; rot = x*cos2 + t
                pps = pspool.tile([128, CH], f32, tag="mm", bufs=2)
                nc.tensor.matmul(
                    pps[:, :], perm[:, :], srd[:, sl],
                    start=True, stop=True,
                )
                tmp = spool.tile([128, CH], bf16, tag="ropetmp", bufs=2)
                nc.vector.tensor_mul(tmp[:, :], pps[:, :], sin2[:, sl])
                nc.vector.tensor_mul(srd[:, sl], srd[:, sl], cos2[:, sl])
                for dst, p0, p1 in dests:
                    nc.vector.tensor_add(
                        dst[p0:p1, sl], srd[p0:p1, sl], tmp[p0:p1, :]
                    )

            def emit_proj(which, dest, dsl, xc, tag="mm"):
                ps = pspool.tile([128, CH], f32, tag=tag, bufs=2)
                for kt in range(8):
                    nc.tensor.matmul(
                        ps[:, :],
                        wqkvT[:, kt * 3 * EPC + which * EPC:
                              kt * 3 * EPC + (which + 1) * EPC],
                        xc[:, kt * CH:(kt + 1) * CH],
                        start=(kt == 0),
                        stop=(kt == 7),
                    )
                nc.vector.tensor_copy(dest[:, dsl], ps[:, :])

            def phase1_pieces(c, xc, xc_next, alone=False):
                # rope is emitted right after each projection so attention
                # chunks never wait on a late rope of the last QKV chunk.
                # Standalone chunks borrow the idle score PSUM ring for the
                # projections so bank recycling never stalls the PE.
                ptag = "sc" if alone else "mm"
                if xc_next is not None:
                    cn = c + 1
                    nc.sync.dma_start(
                        xc_next[:, :], xS_e[cn * 128:(cn + 1) * 128, :]
                    )
                sl = slice(c * CH, (c + 1) * CH)
                emit_proj(1, k_sb, sl, xc, tag=ptag)
                yield
                emit_rope(sl, k_sb, ((k_sb, 0, 128),))
                yield
                emit_proj(0, q_sb, sl, xc, tag=ptag)
                yield
                emit_rope(sl, q_sb, ((qzA, 0, DH), (qzB, DH, 128)))
                yield
                vTc = spool.tile([128, CH], bf16, tag="vTc", bufs=2)
                emit_proj(2, vTc, slice(0, CH), xc, tag=ptag)
                yield
                # transpose v chunk into 130-wide per-slot v tables
                for tt in range(CH // 128):
                    slot = c * (CH // 128) + tt
                    tsl = slice(tt * 128, (tt + 1) * 128)
                    tp = pspool.tile([128, 128], bf16, tag="sc", bufs=2)
                    nc.tensor.transpose(tp[:, :], vTc[:, tsl], ident[:, :])
                    nc.vector.tensor_copy(vt3[:, slot, 0:DH], tp[:, 0:DH])
                    nc.vector.tensor_copy(
                        vt3[:, slot, DH + 1:2 * DH + 1], tp[:, DH:2 * DH]
                    )
                    if tt == 1:
                        yield
                yield

            def drain(gen):
                if gen is not None:
                    for _ in gen:
                        pass

            # ================= Attention machinery ================================
            NKT = N // KT                      # 16 key tiles per chunk
            qzs = (qzA, qzB)

            def emit_pv_pair(st, pair):
                (b, qc, ops, expTs) = st
                for h in range(HPC):
                    for kt in (2 * pair, 2 * pair + 1):
                        slot = b * (N // 128) + kt
                        nc.tensor.matmul(
                            ops[0:DH + 1, h * QC:(h + 1) * QC],
                            vtab[:, slot * VS + h * (DH + 1):
                                 slot * VS + (h + 1) * (DH + 1)],
                            expTs[h][:, kt * QC:(kt + 1) * QC],
                            start=(kt == 0),
                            stop=(kt == NKT - 1),
                        )

            def emit_normalize(st):
                # per-engine batching: both heads' vector prep first, then
                # both broadcasts, then both muls — the vector queue never
                # parks mid-chain waiting on the gpsimd broadcast
                (b, qc, ops, expTs) = st
                q0 = b * N + qc * QC
                recs, bcss = [], []
                for h in range(HPC):
                    hc = h * QC
                    den = spool.tile([1, QC], f32, tag="den", bufs=2)
                    nc.vector.tensor_copy(den[0:1, :], ops[DH:DH + 1, hc:hc + QC])
                    rec = spool.tile([1, QC], f32, tag="rec", bufs=2)
                    nc.vector.reciprocal_approx_fast(rec[0:1, :], den[0:1, :])
                    recs.append(rec)
                for h in range(HPC):
                    bcs = spool.tile([64, QC], f32, tag="bcs", bufs=2)
                    nc.gpsimd.partition_broadcast(bcs[:, :], recs[h][0:1, :])
                    bcss.append(bcs)
                for h in range(HPC):
                    nc.vector.tensor_mul(
                        ovT[h * DH:(h + 1) * DH, q0:q0 + QC],
                        ops[0:DH, h * QC:(h + 1) * QC], bcss[h][:, :]
                    )

            # pieces 0-2: 1024 tokens, core j owns 128-token tile j.
            # piece 3 is split into two half-pieces (3a: tokens 3072-3584,
            # 3b: 3584-4096) with 64-token ownership so 3a's AllToAll flies
            # while chunk 7's own PV is still running.
            a2a_in = [dpool.tile([NCORES * 128, PT // NCORES], bf16,
                                 name=f"a2a_in{p}") for p in range(NPIECE - 1)]
            a2a_out = [dpool.tile([NCORES * 128, PT // NCORES], bf16,
                                  name=f"a2a_out{p}") for p in range(NPIECE - 1)]
            a2a_in_h = [dpool.tile([NCORES * 128, PT // (2 * NCORES)], bf16,
                                   name=f"a2a_in_h{i}") for i in range(2)]
            a2a_out_h = [dpool.tile([NCORES * 128, PT // (2 * NCORES)], bf16,
                                    name=f"a2a_out_h{i}") for i in range(2)]

            def emit_stage(p, j0, j1):
                # stage my features for peers j0..j1's token tiles of piece p
                for j in range(j0, j1):
                    c0 = p * PT + j * 128
                    nc.gpsimd.dma_start(
                        a2a_in[p][j * 128:(j + 1) * 128, :],
                        ovT[:, c0:c0 + 128],
                    )

            def emit_cc(ins_t, outs_t):
                # AllToAll (input split along dim 0, one block per peer)
                nc.gpsimd.collective_compute(
                    "AllToAll",
                    mybir.AluOpType.bypass,
                    ins=[ins_t.opt()],
                    outs=[outs_t.opt()],
                    replica_groups=[list(range(NCORES))],
                )

            def emit_comm(p):
                emit_stage(p, 0, NCORES)
                emit_cc(a2a_in[p], a2a_out[p])

            def emit_comm_half(i):
                # half-piece i of piece 3: 512 tokens, 64-token ownership.
                # staging fans out over three issue rings to cut the serial
                # descriptor-issue latency on the tail critical path
                engs = (nc.gpsimd, nc.sync, nc.scalar)
                for j in range(NCORES):
                    c0 = 3 * PT + i * (PT // 2) + j * 64
                    engs[j % 3].dma_start(
                        a2a_in_h[i][j * 128:(j + 1) * 128, :],
                        ovT[:, c0:c0 + 64],
                    )
                emit_cc(a2a_in_h[i], a2a_out_h[i])

            def outproj_pieces(p):
                # gathers split across the two HWDGE rings for latency
                gT = spool.tile([128, NCORES * 128], bf16, tag="gT", bufs=2)
                for s in range(NCORES):
                    eng = nc.sync if s % 2 == 0 else nc.scalar
                    eng.dma_start(
                        gT[:, s * 128:(s + 1) * 128],
                        a2a_out[p][s * 128:(s + 1) * 128, :],
                    )
                yield
                osb = spool.tile([128, HID], f32, tag="osb", bufs=2)
                for nn in range(HID // 512):
                    odps = pspool.tile([128, 512], f32, tag="mm", bufs=2)
                    for s in range(8):
                        nc.tensor.matmul(
                            odps[:, :],
                            gT[:, s * 128:(s + 1) * 128],
                            woutT[:, s * HID + nn * 512:s * HID + (nn + 1) * 512],
                            start=(s == 0),
                            stop=(s == 7),
                        )
                        if s == 3:
                            yield
                    nc.vector.tensor_copy(osb[:, nn * 512:(nn + 1) * 512], odps[:, :])
                    nc.sync.dma_start(
                        out_e[p * 128:(p + 1) * 128, nn * 512:(nn + 1) * 512],
                        osb[:, nn * 512:(nn + 1) * 512],
                    )
                    yield

            def emit_outproj(p):
                drain(outproj_pieces(p))

            def emit_outproj_half(i):
                # output projection for 512-token half-piece i of piece 3
                gTh = spool.tile([128, NCORES * 64], bf16, tag="gT", bufs=2)
                for s in range(NCORES):
                    eng = nc.sync if s % 2 == 0 else nc.scalar
                    eng.dma_start(
                        gTh[:, s * 64:(s + 1) * 64],
                        a2a_out_h[i][s * 128:(s + 1) * 128, :],
                    )
                osb = spool.tile([128, HID], f32, tag="osb", bufs=2)
                for nn in range(HID // 512):
                    odps = pspool.tile([DH, 512], f32, tag="mm", bufs=2)
                    for s in range(8):
                        nc.tensor.matmul(
                            odps[:, :],
                            gTh[:, s * 64:(s + 1) * 64],
                            woutT[:, s * HID + nn * 512:s * HID + (nn + 1) * 512],
                            start=(s == 0),
                            stop=(s == 7),
                        )
                    nc.vector.tensor_copy(
                        osb[0:DH, nn * 512:(nn + 1) * 512], odps[:, :])
                    nc.sync.dma_start(
                        out_e[3 * 128 + i * 64:3 * 128 + (i + 1) * 64,
                              nn * 512:(nn + 1) * 512],
                        osb[0:DH, nn * 512:(nn + 1) * 512],
                    )

            st = {"pending": None}

            def attn_chunk(ci, filler=None, self_pv=False, pe_filler=None,
                           piece_done=None, pv_early=False):
                # pending PV runs 2 pairs per score-pair in the first half of
                # the loop so normalize lands mid-chunk; piece_done() is
                # called right after it (comm staging goes out half a chunk
                # earlier than waiting for the chunk end)
                b, qc = divmod(ci, N // QC)
                q0 = b * N + qc * QC
                expTs = (spool.tile([128, NKT * QC], bf16, name="expTA",
                                    tag="expTA", bufs=2),
                         spool.tile([128, NKT * QC], bf16, name="expTB",
                                    tag="expTB", bufs=2))
                nxt_ops = None
                for pair in range(NKT // 2):
                    # independent PE work (pending PV, filler) goes FIRST so
                    # the in-order PE queue never parks on a score psum slot
                    # still being read by the trailing exp stream
                    if st["pending"] is not None:
                        if pv_early:
                            if pair < NKT // 4:
                                emit_pv_pair(st["pending"], 2 * pair)
                                emit_pv_pair(st["pending"], 2 * pair + 1)
                            elif pair == NKT // 4:
                                emit_normalize(st["pending"])
                                if piece_done is not None:
                                    piece_done()
                        else:
                            emit_pv_pair(st["pending"], pair)
                    if filler is not None:
                        next(filler, None)
                    for h in range(HPC):
                        sps = pspool.tile([128, 2 * QC], f32, tag="sc", bufs=2)
                        for half in range(2):
                            k0 = b * N + (2 * pair + half) * KT
                            nc.tensor.matmul(
                                sps[:, half * QC:(half + 1) * QC],
                                k_sb[:, k0:k0 + KT],
                                qzs[h][:, q0:q0 + QC],
                                start=True, stop=True,
                            )
                        nc.scalar.activation(
                            expTs[h][:, 2 * pair * QC:(2 * pair + 2) * QC],
                            sps[:, :],
                            mybir.ActivationFunctionType.Exp,
                            scale=DH ** -0.5,
                        )

                if st["pending"] is not None and not pv_early:
                    emit_normalize(st["pending"])
                    if piece_done is not None:
                        piece_done()
                if filler is not None:
                    drain(filler)
                ops = pspool.tile([128, 2 * QC], f32, tag="pv", bufs=1)
                nxt = (b, qc, ops, expTs)
                if self_pv:
                    # final chunk: its own PV runs immediately (exps nearly
                    # drained); normalize + the last half-piece comm go out
                    # BEFORE the remaining outproj work so the PE chews
                    # outproj matmuls while the AllToAll is in flight
                    for pair in range(NKT // 2):
                        emit_pv_pair(nxt, pair)
                    emit_normalize(nxt)
                    emit_comm_half(1)
                    if pe_filler is not None:
                        pe_filler()
                    st["pending"] = None
                else:
                    st["pending"] = nxt

            # ================= Top-level schedule ================================
            # CC-stream warmup: two tiny collectives with no dependencies,
            # triggered immediately — the collectives runtime takes a
            # variable 40-100us to come alive, so the first REAL comm must
            # not be the op that pays for it (its payload is never read).
            ccw_in = dpool.tile([NCORES, 128], bf16, name="ccw_in")
            ccw_out = dpool.tile([NCORES, 128], bf16, name="ccw_out")
            emit_cc(ccw_in, ccw_out)

            xcs = [spool.tile([128, 8 * CH], bf16, tag="xc", bufs=3,
                              name=f"xc{c}") for c in range(NCH)]
            # chunk 0 in halves: the K projection's first 4 k-tile matmuls
            # can start once the first half (and wqkv half) land
            nc.sync.dma_start(xcs[0][:, 0:4 * CH], xS_e[0:128, 0:4 * CH])
            nc.sync.dma_start(xcs[0][:, 4 * CH:], xS_e[0:128, 4 * CH:])
            # rope factors arrive under chunk-0/1 compute on the scalar ring;
            # batch-0 halves first (chunk-0 rope runs early in the chunk)
            nc.scalar.dma_start(sin2[:, 0:T // 2], sin2_e[:, 0:T // 2])
            nc.scalar.dma_start(cos2[:, 0:T // 2], cos2_e[:, 0:T // 2])
            nc.scalar.dma_start(sin2[:, T // 2:], sin2_e[:, T // 2:])
            nc.scalar.dma_start(cos2[:, T // 2:], cos2_e[:, T // 2:])

            for c in range(4):
                drain(phase1_pieces(c, xcs[c], xcs[c + 1]))
                if c == 0:
                    emit_cc(ccw_in, ccw_out)
                    nc.scalar.dma_start(woutT[:, :], woutS_e[:, :])
            for i in range(4):
                xnx = xcs[i + 5] if i + 5 < NCH else None
                attn_chunk(i, filler=phase1_pieces(4 + i, xcs[4 + i], xnx),
                           piece_done=(lambda: emit_comm(0)) if i == 2 else None)
            attn_chunk(4, piece_done=lambda: emit_comm(1))
            attn_chunk(5, filler=outproj_pieces(0))
            attn_chunk(6, filler=outproj_pieces(1),
                       piece_done=lambda: emit_comm(2))
            attn_chunk(7, self_pv=True,
                       pe_filler=lambda: emit_outproj(2),
                       piece_done=lambda: emit_comm_half(0))
            emit_outproj_half(0)
            emit_outproj_half(1)

    nc.finalize()
    return nc


def _host_inputs(x, rope, Wqkv, Wout):
    """Build the 8 per-core input maps with host-side layout prep."""
    xf = np.ascontiguousarray(x.reshape(T, HID).T).astype(_bf16)        # [1024, 4096]
    # swizzle so chunk c is one contiguous [128, 4096] block:
    # xS[c*128+p, kt*512+t] = xf[kt*128+p, c*512+t]
    xS = np.ascontiguousarray(
        xf.reshape(8, 128, 8, 512).transpose(2, 1, 0, 3).reshape(HID, T))
    woutT = np.ascontiguousarray(Wout.T).astype(_bf16)                  # [1024, 1024]
    woutS = np.ascontiguousarray(
        woutT.reshape(8, 128, HID).transpose(1, 0, 2).reshape(128, 8 * HID))

    rf = rope.reshape(T, DH)                                            # [4096, 64]
    cosE = np.repeat(rf[:, 0::2], 2, axis=1).T                          # [64, 4096]
    sinE = np.repeat(rf[:, 1::2], 2, axis=1).T
    sgn = np.where(np.arange(DH) % 2 == 0, -1.0, 1.0)[:, None]
    sinS = (sinE * sgn)
    cos2 = np.ascontiguousarray(np.concatenate([cosE, cosE], 0)).astype(_bf16)
    sin2 = np.ascontiguousarray(np.concatenate([sinS, sinS], 0)).astype(_bf16)

    pm = np.zeros((128, 128), np.float32)
    for d in range(128):
        pm[d ^ 1, d] = 1.0       # partner[d] = q[d^1]; lhsT = S (symmetric)
    permident = np.concatenate(
        [pm.astype(_bf16), np.eye(128, dtype=np.float32).astype(_bf16)], axis=1)
    permident = np.ascontiguousarray(permident)

    w3 = Wqkv.reshape(3, H, DH, HID)
    in_maps = []
    for c in range(NCORES):
        blocks = []
        for which in range(3):
            for hl in range(HPC):
                blocks.append(w3[which, 2 * c + hl])                    # [64, 1024]
        wq = np.concatenate(blocks, 0)                                  # [384, 1024]
        wqkvT = np.ascontiguousarray(wq.T).astype(_bf16)                # [1024, 384]
        wqkvS = np.ascontiguousarray(
            wqkvT.reshape(8, 128, 384).transpose(1, 0, 2).reshape(128, 8 * 384))
        in_maps.append({
            "xS": xS, "wqkvS": wqkvS, "woutS": woutS,
            "cos2": cos2, "sin2": sin2, "permident": permident,
        })
    return in_maps


_CACHE = {}


def kernel(x, rope, Wqkv, Wout):
    from concourse.bass_utils import run_bass_kernel_spmd

    if "nc" not in _CACHE:
        _CACHE["nc"] = _build_graph()
    nc = _CACHE["nc"]
    in_maps = _host_inputs(np.asarray(x, np.float32), np.asarray(rope, np.float32),
                           np.asarray(Wqkv, np.float32), np.asarray(Wout, np.float32))
    res = run_bass_kernel_spmd(nc, in_maps, core_ids=list(range(NCORES)))
    parts = np.stack([np.asarray(res.results[i]["out"], np.float32)
                      for i in range(NCORES)])                          # [8, 512, 1024]
    full = np.empty((T, HID), np.float32)
    # pieces 0-2: core j's rows [p*128:(p+1)*128] hold tokens p*1024 + j*128..
    p012 = parts[:, 0:384].reshape(NCORES, 3, 128, HID).transpose(1, 0, 2, 3)
    full[0:3 * PT] = p012.reshape(3 * PT, HID)
    # piece 3 halves: rows [384+i*64 : 448+i*64) hold tokens
    # 3072 + i*512 + j*64 ..
    for i in range(2):
        ph = parts[:, 384 + i * 64:448 + i * 64]                        # [8, 64, HID]
        full[3 * PT + i * 512:3 * PT + (i + 1) * 512] = ph.reshape(512, HID)
    return full.reshape(B, N, HID)


# revision 74
# speedup vs baseline: 1.0063x; 1.0063x over previous
"""Distributed Bass kernel: fused multi-head attention block on 8 TRN2 NeuronCores.

Problem: x[2,2048,1024] -> QKV proj -> RoPE(q,k) -> softmax(q k^T/8) v -> out proj.

Sharding: tensor-parallel over heads. 16 heads / 8 cores = 2 heads per core.
Each core computes QKV for its 2 heads (full sequence), RoPE, attention, then
chunked AllToAlls (4 pieces of 1024 tokens, cc_dim=Free, pipelined under the
attention compute) convert head-sharding to token-sharding so the output
projection runs against the FULL Wout with no AllReduce. Token ownership is
interleaved per 128-token tile: within piece p core j owns tokens
[p*1024+j*128, p*1024+(j+1)*128). Host reassembles the 4x128-row tiles.

Schedule (v2): all bulk loads are single large DMA descriptors (host
pre-swizzles x/Wqkv/Wout so each is one contiguous block per chunk) issued
from the sync/scalar HWDGE rings instead of gpsimd SWDGE, which cuts the
descriptor-issue preamble from ~20us to ~6us. QKV chunks 4-7 are interleaved
per-PAIR into attention chunks 0-3 so the PE never stalls on the scalar
engine's exp stream. The final chunk's PV pairs interleave with its own exp
stream, with outproj(2) as PE filler, then normalize->stage->AllToAll->
outproj for the last piece runs immediately.

Compute dtype bf16, f32 PSUM accumulation. Softmax skips the max-subtraction
(scores ~N(0,2), exp safe in f32) and folds the denominator into the PV matmul
via a ones-column in the per-head v table ([key,130] slots: vA|1|vB|1).
"""

import sys

for _p in ("/opt/trn_rl_repo", "/root/.axon_site/_ro/trn_rl_repo"):
    if _p not in sys.path:
        sys.path.append(_p)

import numpy as np
import ml_dtypes

B, N, HID = 2, 2048, 1024
H, DH = 16, 64
NCORES = 8
HPC = H // NCORES          # heads per core = 2
T = B * N                  # 4096 flattened tokens
TS = T // NCORES           # 512 tokens per core after AllToAll
EPC = HPC * DH             # 128 features per core
CH = 512                   # token chunk for QKV phase
NCH = T // CH              # 8 chunks
KT = 128                   # key tile
QC = 512                   # query chunk in attention
NPIECE = 4                 # a2a pieces (1024 tokens each)
PT = T // NPIECE           # 1024 tokens per piece
VS = 2 * (DH + 1)          # 130-wide v-table slot: [vA(64) | 1 | vB(64) | 1]

_bf16 = ml_dtypes.bfloat16


def _build_graph():
    import concourse.bass as bass
    import concourse.mybir as mybir
    import concourse.tile as tile
    from concourse import bacc

    f32 = mybir.dt.float32
    bf16 = mybir.dt.bfloat16

    nc = bacc.Bacc("TRN2", target_bir_lowering=False, debug=False, num_devices=NCORES)

    # host pre-swizzled: xS[c*128+p, kt*512+t] = x^T[kt*128+p, c*512+t]
    xS_e = nc.declare_dram_parameter("xS", [HID, T], bf16, isOutput=False)
    # wqkvS[p, kt*384+j] = Wqkv'[kt*128+p, j]
    wqkvS_e = nc.declare_dram_parameter("wqkvS", [128, 8 * 3 * EPC], bf16, isOutput=False)
    # woutS[p, kt*1024+j] = Wout^T[kt*128+p, j]
    woutS_e = nc.declare_dram_parameter("woutS", [128, 8 * HID], bf16, isOutput=False)
    cos2_e = nc.declare_dram_parameter("cos2", [2 * DH, T], bf16, isOutput=False)
    sin2_e = nc.declare_dram_parameter("sin2", [2 * DH, T], bf16, isOutput=False)
    permident_e = nc.declare_dram_parameter("permident", [128, 256], bf16, isOutput=False)
    out_e = nc.declare_dram_parameter("out", [TS, HID], f32, isOutput=True)

    with tile.TileContext(nc) as tc:
        with (
            tc.tile_pool(name="const", bufs=1) as cpool,
            tc.tile_pool(name="work", bufs=1) as wpool,
            tc.tile_pool(name="stream", bufs=4) as spool,
            tc.tile_pool(name="psum", bufs=2, space="PSUM") as pspool,
            tc.tile_pool(name="dram", bufs=1, space="DRAM") as dpool,
        ):
            # ---- constants / weights (few-descriptor loads, HWDGE) ----
            # wqkv in halves so the first QKV matmuls start sooner
            wqkvT = cpool.tile([128, 8 * 3 * EPC], bf16)
            nc.sync.dma_start(wqkvT[:, 0:4 * 3 * EPC], wqkvS_e[:, 0:4 * 3 * EPC])
            nc.sync.dma_start(wqkvT[:, 4 * 3 * EPC:], wqkvS_e[:, 4 * 3 * EPC:])
            permident = cpool.tile([128, 256], bf16)
            nc.scalar.dma_start(permident[:, :], permident_e[:, :])
            perm = permident[:, 0:128]
            ident = permident[:, 128:256]
            woutT = cpool.tile([128, 8 * HID], bf16)
            cos2 = cpool.tile([128, T], bf16)
            sin2 = cpool.tile([128, T], bf16)

            # ---- persistent working tensors ----
            q_sb = wpool.tile([128, T], bf16)      # raw q (rope intermediate)
            k_sb = wpool.tile([128, T], bf16)      # becomes roped k
            qzA = wpool.tile([128, T], bf16)       # roped qA rows 0-63, 0 below
            qzB = wpool.tile([128, T], bf16)       # roped qB rows 64-127, 0 above
            vtab = wpool.tile([128, 32 * VS], bf16)  # [key, vA|1|vB|1] per slot
            ovT = wpool.tile([128, T], bf16)       # attention out ^T

            vt3 = vtab.rearrange("p (s c) -> p s c", c=VS)
            ones64 = cpool.tile([1, DH], bf16)
            nc.vector.memset(ones64[0:1, :], 1.0)
            nc.vector.memset(qzA[DH:128, :], 0.0)
            nc.vector.memset(qzB[0:DH, :], 0.0)
            nc.vector.memset(vt3[:, :, DH:DH + 1], 1.0)
            nc.vector.memset(vt3[:, :, 2 * DH + 1:2 * DH + 2], 1.0)

            # ================= Phase 1 chunk: QKV + RoPE + v-transpose ============
            # emitted as a generator of "pieces" so attention chunks can
            # interleave QKV work into their pair loops (fills exp-wait gaps)
            def emit_rope(sl, srd, dests):
                # RoPE: t = P@x * sin2 ; rot = x*cos2 + t
                pps = pspool.tile([128, CH], f32, tag="mm", bufs=2)
                nc.tensor.matmul(
                    pps[:, :], perm[:, :], srd[:, sl],
                    start=True, stop=True,
                )
                tmp = spool.tile([128, CH], bf16, tag="ropetmp", bufs=2)
                nc.vector.tensor_mul(tmp[:, :], pps[:, :], sin2[:, sl])
                nc.vector.tensor_mul(srd[:, sl], srd[:, sl], cos2[:, sl])
                for dst, p0, p1 in dests:
                    nc.vector.tensor_add(
                        dst[p0:p1, sl], srd[p0:p1, sl], tmp[p0:p1, :]
                    )

            def emit_proj(which, dest, dsl, xc, tag="mm"):
                ps = pspool.tile([128, CH], f32, tag=tag, bufs=2)
                for kt in range(8):
                    nc.tensor.matmul(
                        ps[:, :],
                        wqkvT[:, kt * 3 * EPC + which * EPC:
                              kt * 3 * EPC + (which + 1) * EPC],
                        xc[:, kt * CH:(kt + 1) * CH],
                        start=(kt == 0),
                        stop=(kt == 7),
                    )
                nc.vector.tensor_copy(dest[:, dsl], ps[:, :])

            def phase1_pieces(c, xc, xc_next, alone=False):
                # rope is emitted right after each projection so attention
                # chunks never wait on a late rope of the last QKV chunk.
                # Standalone chunks borrow the idle score PSUM ring for the
                # projections so bank recycling never stalls the PE.
                ptag = "sc" if alone else "mm"
                if xc_next is not None:
                    cn = c + 1
                    nc.sync.dma_start(
                        xc_next[:, :], xS_e[cn * 128:(cn + 1) * 128, :]
                    )
                sl = slice(c * CH, (c + 1) * CH)
                emit_proj(1, k_sb, sl, xc, tag=ptag)
                yield
                emit_rope(sl, k_sb, ((k_sb, 0, 128),))
                yield
                emit_proj(0, q_sb, sl, xc, tag=ptag)
                yield
                emit_rope(sl, q_sb, ((qzA, 0, DH), (qzB, DH, 128)))
                yield
                vTc = spool.tile([128, CH], bf16, tag="vTc", bufs=2)
                emit_proj(2, vTc, slice(0, CH), xc, tag=ptag)
                yield
                # transpose v chunk into 130-wide per-slot v tables
                for tt in range(CH // 128):
                    slot = c * (CH // 128) + tt
                    tsl = slice(tt * 128, (tt + 1) * 128)
                    tp = pspool.tile([128, 128], bf16, tag="sc", bufs=2)
                    nc.tensor.transpose(tp[:, :], vTc[:, tsl], ident[:, :])
                    nc.vector.tensor_copy(vt3[:, slot, 0:DH], tp[:, 0:DH])
                    nc.vector.tensor_copy(
                        vt3[:, slot, DH + 1:2 * DH + 1], tp[:, DH:2 * DH]
                    )
                    if tt == 1:
                        yield
                yield

            def drain(gen):
                if gen is not None:
                    for _ in gen:
                        pass

            # ================= Attention machinery ================================
            NKT = N // KT                      # 16 key tiles per chunk
            qzs = (qzA, qzB)

            def emit_pv_pair(st, pair):
                (b, qc, ops, expTs) = st
                for h in range(HPC):
                    for kt in (2 * pair, 2 * pair + 1):
                        slot = b * (N // 128) + kt
                        nc.tensor.matmul(
                            ops[0:DH + 1, h * QC:(h + 1) * QC],
                            vtab[:, slot * VS + h * (DH + 1):
                                 slot * VS + (h + 1) * (DH + 1)],
                            expTs[h][:, kt * QC:(kt + 1) * QC],
                            start=(kt == 0),
                            stop=(kt == NKT - 1),
                        )

            def emit_normalize(st):
                # per-engine batching: both heads' vector prep first, then
                # both broadcasts, then both muls — the vector queue never
                # parks mid-chain waiting on the gpsimd broadcast
                (b, qc, ops, expTs) = st
                q0 = b * N + qc * QC
                recs, bcss = [], []
                for h in range(HPC):
                    hc = h * QC
                    den = spool.tile([1, QC], f32, tag="den", bufs=2)
                    nc.vector.tensor_copy(den[0:1, :], ops[DH:DH + 1, hc:hc + QC])
                    rec = spool.tile([1, QC], f32, tag="rec", bufs=2)
                    nc.vector.reciprocal_approx_fast(rec[0:1, :], den[0:1, :])
                    recs.append(rec)
                for h in range(HPC):
                    bcs = spool.tile([64, QC], f32, tag="bcs", bufs=2)
                    nc.gpsimd.partition_broadcast(bcs[:, :], recs[h][0:1, :])
                    bcss.append(bcs)
                for h in range(HPC):
                    nc.vector.tensor_mul(
                        ovT[h * DH:(h + 1) * DH, q0:q0 + QC],
                        ops[0:DH, h * QC:(h + 1) * QC], bcss[h][:, :]
                    )

            # pieces 0-2: 1024 tokens, core j owns 128-token tile j.
            # piece 3 is split into two half-pieces (3a: tokens 3072-3584,
            # 3b: 3584-4096) with 64-token ownership so 3a's AllToAll flies
            # while chunk 7's own PV is still running.
            a2a_in = [dpool.tile([NCORES * 128, PT // NCORES], bf16,
                                 name=f"a2a_in{p}") for p in range(NPIECE - 1)]
            a2a_out = [dpool.tile([NCORES * 128, PT // NCORES], bf16,
                                  name=f"a2a_out{p}") for p in range(NPIECE - 1)]
            a2a_in_h = [dpool.tile([NCORES * 128, PT // (2 * NCORES)], bf16,
                                   name=f"a2a_in_h{i}") for i in range(2)]
            a2a_out_h = [dpool.tile([NCORES * 128, PT // (2 * NCORES)], bf16,
                                    name=f"a2a_out_h{i}") for i in range(2)]

            def emit_stage(p, j0, j1):
                # stage my features for peers j0..j1's token tiles of piece p
                for j in range(j0, j1):
                    c0 = p * PT + j * 128
                    nc.gpsimd.dma_start(
                        a2a_in[p][j * 128:(j + 1) * 128, :],
                        ovT[:, c0:c0 + 128],
                    )

            def emit_cc(ins_t, outs_t):
                # AllToAll (input split along dim 0, one block per peer)
                nc.gpsimd.collective_compute(
                    "AllToAll",
                    mybir.AluOpType.bypass,
                    ins=[ins_t.opt()],
                    outs=[outs_t.opt()],
                    replica_groups=[list(range(NCORES))],
                )

            def emit_comm(p):
                emit_stage(p, 0, NCORES)
                emit_cc(a2a_in[p], a2a_out[p])

            def emit_comm_half(i):
                # half-piece i of piece 3: 512 tokens, 64-token ownership.
                # staging fans out over three issue rings to cut the serial
                # descriptor-issue latency on the tail critical path
                engs = (nc.gpsimd, nc.sync, nc.scalar)
                for j in range(NCORES):
                    c0 = 3 * PT + i * (PT // 2) + j * 64
                    engs[j % 3].dma_start(
                        a2a_in_h[i][j * 128:(j + 1) * 128, :],
                        ovT[:, c0:c0 + 64],
                    )
                emit_cc(a2a_in_h[i], a2a_out_h[i])

            def outproj_pieces(p):
                # gathers split across the two HWDGE rings for latency
                gT = spool.tile([128, NCORES * 128], bf16, tag="gT", bufs=2)
                for s in range(NCORES):
                    eng = nc.sync if s % 2 == 0 else nc.scalar
                    eng.dma_start(
                        gT[:, s * 128:(s + 1) * 128],
                        a2a_out[p][s * 128:(s + 1) * 128, :],
                    )
                yield
                osb = spool.tile([128, HID], f32, tag="osb", bufs=2)
                for nn in range(HID // 512):
                    odps = pspool.tile([128, 512], f32, tag="mm", bufs=2)
                    for s in range(8):
                        nc.tensor.matmul(
                            odps[:, :],
                            gT[:, s * 128:(s + 1) * 128],
                            woutT[:, s * HID + nn * 512:s * HID + (nn + 1) * 512],
                            start=(s == 0),
                            stop=(s == 7),
                        )
                        if s == 3:
                            yield
                    nc.vector.tensor_copy(osb[:, nn * 512:(nn + 1) * 512], odps[:, :])
                    nc.sync.dma_start(
                        out_e[p * 128:(p + 1) * 128, nn * 512:(nn + 1) * 512],
                        osb[:, nn * 512:(nn + 1) * 512],
                    )
                    yield

            def emit_outproj(p):
                drain(outproj_pieces(p))

            def emit_outproj_half(i):
                # output projection for 512-token half-piece i of piece 3
                gTh = spool.tile([128, NCORES * 64], bf16, tag="gT", bufs=2)
                for s in range(NCORES):
                    eng = nc.sync if s % 2 == 0 else nc.scalar
                    eng.dma_start(
                        gTh[:, s * 64:(s + 1) * 64],
                        a2a_out_h[i][s * 128:(s + 1) * 128, :],
                    )
                osb = spool.tile([128, HID], f32, tag="osb", bufs=2)
                for nn in range(HID // 512):
                    odps = pspool.tile([DH, 512], f32, tag="mm", bufs=2)
                    for s in range(8):
                        nc.tensor.matmul(
                            odps[:, :],
                            gTh[:, s * 64:(s + 1) * 64],
                            woutT[:, s * HID + nn * 512:s * HID + (nn + 1) * 512],
                            start=(s == 0),
                            stop=(s == 7),
                        )
                    nc.vector.tensor_copy(
                        osb[0:DH, nn * 512:(nn + 1) * 512], odps[:, :])
                    nc.sync.dma_start(
                        out_e[3 * 128 + i * 64:3 * 128 + (i + 1) * 64,
                              nn * 512:(nn + 1) * 512],
                        osb[0:DH, nn * 512:(nn + 1) * 512],
                    )

            st = {"pending": None}

            def attn_chunk(ci, filler=None, self_pv=False, pe_filler=None,
                           piece_done=None, pv_early=False):
                # pending PV runs 2 pairs per score-pair in the first half of
                # the loop so normalize lands mid-chunk; piece_done() is
                # called right after it (comm staging goes out half a chunk
                # earlier than waiting for the chunk end)
                b, qc = divmod(ci, N // QC)
                q0 = b * N + qc * QC
                expTs = (spool.tile([128, NKT * QC], bf16, name="expTA",
                                    tag="expTA", bufs=2),
                         spool.tile([128, NKT * QC], bf16, name="expTB",
                                    tag="expTB", bufs=2))
                nxt_ops = None
                for pair in range(NKT // 2):
                    # independent PE work (pending PV, filler) goes FIRST so
                    # the in-order PE queue never parks on a score psum slot
                    # still being read by the trailing exp stream
                    if st["pending"] is not None:
                        if pv_early:
                            if pair < NKT // 4:
                                emit_pv_pair(st["pending"], 2 * pair)
                                emit_pv_pair(st["pending"], 2 * pair + 1)
                            elif pair == NKT // 4:
                                emit_normalize(st["pending"])
                                if piece_done is not None:
                                    piece_done()
                        else:
                            emit_pv_pair(st["pending"], pair)
                    if filler is not None:
                        next(filler, None)
                    for h in range(HPC):
                        sps = pspool.tile([128, 2 * QC], f32, tag="sc", bufs=2)
                        for half in range(2):
                            k0 = b * N + (2 * pair + half) * KT
                            nc.tensor.matmul(
                                sps[:, half * QC:(half + 1) * QC],
                                k_sb[:, k0:k0 + KT],
                                qzs[h][:, q0:q0 + QC],
                                start=True, stop=True,
                            )
                        nc.scalar.activation(
                            expTs[h][:, 2 * pair * QC:(2 * pair + 2) * QC],
                            sps[:, :],
                            mybir.ActivationFunctionType.Exp,
                            scale=DH ** -0.5,
                        )

                if st["pending"] is not None and not pv_early:
                    emit_normalize(st["pending"])
                    if piece_done is not None:
                        piece_done()
                if filler is not None:
                    drain(filler)
                ops = pspool.tile([128, 2 * QC], f32, tag="pv", bufs=1)
                nxt = (b, qc, ops, expTs)
                if self_pv:
                    # final chunk: its own PV runs immediately (exps nearly
                    # drained); normalize + the last half-piece comm go out
                    # BEFORE the remaining outproj work so the PE chews
                    # outproj matmuls while the AllToAll is in flight
                    for pair in range(NKT // 2):
                        emit_pv_pair(nxt, pair)
                    emit_normalize(nxt)
                    emit_comm_half(1)
                    if pe_filler is not None:
                        pe_filler()
                    st["pending"] = None
                else:
                    st["pending"] = nxt

            # ================= Top-level schedule ================================
            # CC-stream warmup: two tiny collectives with no dependencies,
            # triggered immediately — the collectives runtime takes a
            # variable 40-100us to come alive, so the first REAL comm must
            # not be the op that pays for it (its payload is never read).
            ccw_in = dpool.tile([NCORES, 128], bf16, name="ccw_in")
            ccw_out = dpool.tile([NCORES, 128], bf16, name="ccw_out")
            emit_cc(ccw_in, ccw_out)

            xcs = [spool.tile([128, 8 * CH], bf16, tag="xc", bufs=3,
                              name=f"xc{c}") for c in range(NCH)]
            # chunk 0 in halves: the K projection's first 4 k-tile matmuls
            # can start once the first half (and wqkv half) land
            nc.sync.dma_start(xcs[0][:, 0:4 * CH], xS_e[0:128, 0:4 * CH])
            nc.sync.dma_start(xcs[0][:, 4 * CH:], xS_e[0:128, 4 * CH:])
            # rope factors arrive under chunk-0/1 compute on the scalar ring;
            # batch-0 halves first (chunk-0 rope runs early in the chunk)
            nc.scalar.dma_start(sin2[:, 0:T // 2], sin2_e[:, 0:T // 2])
            nc.scalar.dma_start(cos2[:, 0:T // 2], cos2_e[:, 0:T // 2])
            nc.scalar.dma_start(sin2[:, T // 2:], sin2_e[:, T // 2:])
            nc.scalar.dma_start(cos2[:, T // 2:], cos2_e[:, T // 2:])

            for c in range(4):
                drain(phase1_pieces(c, xcs[c], xcs[c + 1]))
                if c == 0:
                    emit_cc(ccw_in, ccw_out)
                    nc.scalar.dma_start(woutT[:, :], woutS_e[:, :])
            for i in range(4):
                xnx = xcs[i + 5] if i + 5 < NCH else None
                attn_chunk(i, filler=phase1_pieces(4 + i, xcs[4 + i], xnx),
                           piece_done=(lambda: emit_comm(0)) if i == 2 else None)
            attn_chunk(4, piece_done=lambda: emit_comm(1))
            attn_chunk(5, filler=outproj_pieces(0))
            attn_chunk(6, filler=outproj_pieces(1),
                       piece_done=lambda: emit_comm(2))
            attn_chunk(7, self_pv=True, pv_early=True,
                       pe_filler=lambda: emit_outproj(2),
                       piece_done=lambda: emit_comm_half(0))
            emit_outproj_half(0)
            emit_outproj_half(1)

    nc.finalize()
    return nc


def _host_inputs(x, rope, Wqkv, Wout):
    """Build the 8 per-core input maps with host-side layout prep."""
    xf = np.ascontiguousarray(x.reshape(T, HID).T).astype(_bf16)        # [1024, 4096]
    # swizzle so chunk c is one contiguous [128, 4096] block:
    # xS[c*128+p, kt*512+t] = xf[kt*128+p, c*512+t]
    xS = np.ascontiguousarray(
        xf.reshape(8, 128, 8, 512).transpose(2, 1, 0, 3).reshape(HID, T))
    woutT = np.ascontiguousarray(Wout.T).astype(_bf16)                  # [1024, 1024]
    woutS = np.ascontiguousarray(
        woutT.reshape(8, 128, HID).transpose(1, 0, 2).reshape(128, 8 * HID))

    rf = rope.reshape(T, DH)                                            # [4096, 64]
    cosE = np.repeat(rf[:, 0::2], 2, axis=1).T                          # [64, 4096]
    sinE = np.repeat(rf[:, 1::2], 2, axis=1).T
    sgn = np.where(np.arange(DH) % 2 == 0, -1.0, 1.0)[:, None]
    sinS = (sinE * sgn)
    cos2 = np.ascontiguousarray(np.concatenate([cosE, cosE], 0)).astype(_bf16)
    sin2 = np.ascontiguousarray(np.concatenate([sinS, sinS], 0)).astype(_bf16)

    pm = np.zeros((128, 128), np.float32)
    for d in range(128):
        pm[d ^ 1, d] = 1.0       # partner[d] = q[d^1]; lhsT = S (symmetric)
    permident = np.concatenate(
        [pm.astype(_bf16), np.eye(128, dtype=np.float32).astype(_bf16)], axis=1)
    permident = np.ascontiguousarray(permident)

    w3 = Wqkv.reshape(3, H, DH, HID)
    in_maps = []
    for c in range(NCORES):
        blocks = []
        for which in range(3):
            for hl in range(HPC):
                blocks.append(w3[which, 2 * c + hl])                    # [64, 1024]
        wq = np.concatenate(blocks, 0)                                  # [384, 1024]
        wqkvT = np.ascontiguousarray(wq.T).astype(_bf16)                # [1024, 384]
        wqkvS = np.ascontiguousarray(
            wqkvT.reshape(8, 128, 384).transpose(1, 0, 2).reshape(128, 8 * 384))
        in_maps.append({
            "xS": xS, "wqkvS": wqkvS, "woutS": woutS,
            "cos2": cos2, "sin2": sin2, "permident": permident,
        })
    return in_maps


_CACHE = {}


def kernel(x, rope, Wqkv, Wout):
    from concourse.bass_utils import run_bass_kernel_spmd

    if "nc" not in _CACHE:
        _CACHE["nc"] = _build_graph()
    nc = _CACHE["nc"]
    in_maps = _host_inputs(np.asarray(x, np.float32), np.asarray(rope, np.float32),
                           np.asarray(Wqkv, np.float32), np.asarray(Wout, np.float32))
    res = run_bass_kernel_spmd(nc, in_maps, core_ids=list(range(NCORES)))
    parts = np.stack([np.asarray(res.results[i]["out"], np.float32)
                      for i in range(NCORES)])                          # [8, 512, 1024]
    full = np.empty((T, HID), np.float32)
    # pieces 0-2: core j's rows [p*128:(p+1)*128] hold tokens p*1024 + j*128..
    p012 = parts[:, 0:384].reshape(NCORES, 3, 128, HID).transpose(1, 0, 2, 3)
    full[0:3 * PT] = p012.reshape(3 * PT, HID)
    # piece 3 halves: rows [384+i*64 : 448+i*64) hold tokens
    # 3072 + i*512 + j*64 ..
    for i in range(2):
        ph = parts[:, 384 + i * 64:448 + i * 64]                        # [8, 64, HID]
        full[3 * PT + i * 512:3 * PT + (i + 1) * 512] = ph.reshape(512, HID)
    return full.reshape(B, N, HID)
